# revision 24
# baseline (speedup 1.0000x reference)
"""Trainium2 Bass kernel for DifferentiablePointMassSimulator.

Math: the 2-D point-mass scan is reformulated in polar velocity coordinates.
With v = r*e^{i*theta}, a_t = DT*thrust, b_t = DT*torque:
    v' = e^{i*theta} * (r + a + i*b)
so the radius obeys a scalar recurrence independent of the angle:
    m_{t+1} = (m_t + (a^2+b^2)_t) + (2*a_t)*r_t,   r_t = sqrt(m_t)
and the angle increment delta_t = atan2(b_t, r_t + a_t) is computed post-hoc
from the radius sequence with the quarter-angle identity
    delta = 4*atan( b / (h + w1) ),  w1 = u + r',  u = r_t + a_t,  r' = r_{t+1}
    h = sqrt(2 * r' * w1)
whose atan argument always lies in [-1, 1] (ScalarE Arctan domain).
Near the delta ~ +-pi line (u < 0, |b| << |u|) the direct w1 = u + r' suffers
catastrophic cancellation; there we use the exact rationalization
    w1 = b^2 / (r' - u)        (since r'^2 - u^2 = b^2)
selected with copy_predicated on (u < 0).
theta_t = theta0 + cumsum(delta) via tensor_tensor_scan; sin/cos via floor-mod
(AluOpType.mod) + add_range_wrap range reduction into [-pi, pi], evaluated with
the ScalarE Sin table (cos via the +pi/2 shift inside add_range_wrap).
Positions: pos_{t+1} = pos_t + DT*(v_t + v_{t+1})/2 exactly, so with
vxs_t = DT*vx_out[t]:
    px_out[t] = Cx_t - 0.5*vxs_t,  Cx = scan(+, vxs, init = px0 + DT*vx0/2).

Sharding: pure data parallel, batch 16384 -> 8 cores x 2048; on-core layout
batch = 128 partitions x 16 columns (b_local = p*16 + col).
"""

import sys

sys.path.insert(0, "/opt/trn_rl_repo")

import numpy as np

import concourse.bass as bass
import concourse.mybir as mybir
from concourse.tile import TileContext

DT = 1.0 / 30.0
P = 128          # partitions
NB = 16          # batch columns per partition
H = 256          # horizon
HP = H + 1
S = 8            # state dim
BC = P * NB      # batch per core (2048)
NCORES = 8
B = BC * NCORES

F32 = mybir.dt.float32
PI = float(np.pi)
TWO_PI = float(2.0 * np.pi)

_BUILT = None


def build_nc(fixups=True):
    Alu = mybir.AluOpType
    AF = mybir.ActivationFunctionType

    nc = bass.Bass()
    ist = nc.dram_tensor("initial_state", [BC, S], F32, kind="ExternalInput")
    act = nc.dram_tensor("actions", [BC, H, 2], F32, kind="ExternalInput")
    traj = nc.dram_tensor("traj", [BC, H, S], F32, kind="ExternalOutput")

    ist_r = ist.rearrange("(p q) s -> p (q s)", p=P)       # (128, 128)
    act_r = act.rearrange("(p q) h a -> p (q h a)", p=P)   # (128, 8192)
    traj_r = traj.rearrange("(p q) h s -> p (q h s)", p=P)  # (128, 32768)

    v = nc.vector
    g = nc.gpsimd
    sc = nc.scalar
    sy = nc.sync

    with TileContext(nc) as tc:
        with tc.tile_pool(name="pers", bufs=1) as pp, \
                tc.tile_pool(name="outc", bufs=2) as op:
            RP = pp.tile([P, NB * HP], F32, tag="RP")      # r_k at slot k
            A2 = pp.tile([P, NB * H], F32, tag="A2")       # 2*DT*thrust
            BQ = pp.tile([P, NB * H], F32, tag="BQ")       # DT*torque
            CARR = pp.tile([P, NB * H], F32, tag="CARR")   # a^2+b^2
            IS = pp.tile([P, NB * S], F32, tag="IS")
            # big tmps: 3 explicit rotating slots
            S1 = pp.tile([P, NB * H], F32, tag="S1")
            S2 = pp.tile([P, NB * H], F32, tag="S2")
            S3 = pp.tile([P, NB * H], F32, tag="S3")
            # small state tiles, packed into one allocation
            SMALL = pp.tile([P, NB * 12], F32, tag="SMALL")
            M = SMALL[:, 0 * NB:1 * NB]
            T1 = SMALL[:, 1 * NB:2 * NB]
            GA = SMALL[:, 2 * NB:3 * NB]   # scan scratch half 0
            GB = SMALL[:, 3 * NB:4 * NB]   # scan scratch half 1
            Q0 = SMALL[:, 4 * NB:5 * NB]
            A0 = SMALL[:, 5 * NB:6 * NB]
            KX = SMALL[:, 6 * NB:7 * NB]
            KY = SMALL[:, 7 * NB:8 * NB]
            W10 = SMALL[:, 8 * NB:9 * NB]
            RMU0 = SMALL[:, 9 * NB:10 * NB]
            MSK0 = SMALL[:, 10 * NB:11 * NB]

            # multi-dim views
            IS3 = IS.rearrange("p (b s) -> p b s", b=NB)
            RP3 = RP.rearrange("p (b k) -> p b k", b=NB)
            A23 = A2.rearrange("p (b t) -> p b t", b=NB)
            BQ3 = BQ.rearrange("p (b t) -> p b t", b=NB)
            C3 = CARR.rearrange("p (b t) -> p b t", b=NB)

            px0 = IS3[:, :, 0]
            py0 = IS3[:, :, 1]
            vx0 = IS3[:, :, 2]
            vy0 = IS3[:, :, 3]

            # ---------------- phase 0: loads + precompute ----------------
            sy.dma_start(out=IS[:], in_=ist_r[:])

            # actions -> A2, BQ, CARR (two 2MB chunks; squares on ScalarE)
            for hb in range(2):
                chunk = pp.tile([P, 8 * H * 2], F32, tag="S1" if hb == 0 else "S2")
                sy.dma_start(
                    out=chunk[:], in_=act_r[:, hb * 4096:(hb + 1) * 4096]
                )
                ch = chunk.rearrange("p (b t a) -> p b t a", b=8, t=H)
                thr = ch[:, :, :, 0]
                tor = ch[:, :, :, 1]
                bsl = slice(hb * 8, (hb + 1) * 8)
                v.tensor_scalar(A23[:, bsl, :], thr, 2.0 * DT, None, Alu.mult)
                v.tensor_scalar(BQ3[:, bsl, :], tor, DT, None, Alu.mult)
                sq = pp.tile([P, 8 * H], F32, tag="S3")
                sq3 = sq.rearrange("p (b t) -> p b t", b=8)
                sc.activation(sq3, thr, AF.Square, scale=DT)   # (DT*T)^2
                sq2 = pp.tile([P, 8 * H], F32, tag="S1" if hb == 1 else "S2")
                sq23 = sq2.rearrange("p (b t) -> p b t", b=8)
                sc.activation(sq23, tor, AF.Square, scale=DT)  # (DT*Q)^2
                v.tensor_add(C3[:, bsl, :], sq3, sq23)

            # r0, m0
            sc.activation(GA, vx0, AF.Square)
            sc.activation(GB, vy0, AF.Square)
            v.tensor_add(M, GA, GB)                      # m0 = r0^2
            sc.activation(RP3[:, :, 0], M, AF.Sqrt)      # r0
            r0 = RP3[:, :, 0]

            # theta0/4 prep: w10 = r0 + vx0, rationalized to vy0^2/(r0 - vx0)
            # when vx0 < 0.  All reciprocals are deferred to the ln/exp table
            # section after the scan (no custom DVE ops available).
            v.tensor_add(W10, r0, vx0)                   # w10 direct
            v.tensor_sub(RMU0, r0, vx0)                  # r0 - vx0
            MSK0i = MSK0.bitcast(mybir.dt.int32)
            v.tensor_scalar(MSK0i, vx0, 0.0, None, Alu.is_lt)  # mask vx0 < 0

            # pos cumsum seeds
            v.scalar_tensor_tensor(KX, vx0, DT / 2.0, px0, Alu.mult, Alu.add)
            v.scalar_tensor_tensor(KY, vy0, DT / 2.0, py0, Alu.mult, Alu.add)

            # ---------------- phase 1: radius scan ----------------
            # m' = (m + c_t) + (2 a_t) * r_t ; r_{t+1} = sqrt(m')
            # two staggered halves so ScalarE sqrt overlaps VectorE updates
            halves = [slice(0, 8), slice(8, 16)]
            Mh = [M[:, 0:8], M[:, 8:16]]
            T1h = [T1[:, 0:8], T1[:, 8:16]]
            Gh = [GA[:, 0:8], GB[:, 0:8]]
            for t in range(H):
                for hf in (0, 1):
                    v.tensor_add(T1h[hf], Mh[hf], C3[:, halves[hf], t])
                for hf in (0, 1):
                    v.tensor_mul(Gh[hf], A23[:, halves[hf], t], RP3[:, halves[hf], t])
                    v.tensor_add(Mh[hf], T1h[hf], Gh[hf])
                    sc.activation(RP3[:, halves[hf], t + 1], Mh[hf], AF.Sqrt)

            # ---------------- phase 2: angles, velocities, positions ------
            Rsh = RP3[:, :, 0:H]     # r_t
            Rpo = RP3[:, :, 1:HP]    # r_{t+1}
            S1_3 = S1.rearrange("p (b t) -> p b t", b=NB)
            S2_3 = S2.rearrange("p (b t) -> p b t", b=NB)
            S3_3 = S3.rearrange("p (b t) -> p b t", b=NB)

            # A-section: u, w1, w2, h, den, rden, q; w1 rationalized to
            # b^2/(r'-u) where u<0 (exact identity r'^2-u^2=b^2) to avoid
            # catastrophic cancellation near delta ~ +-pi.
            v.scalar_tensor_tensor(S1_3, A23, 0.5, Rsh, Alu.mult, Alu.add)   # u -> S1
            v.tensor_add(S2_3, S1_3, Rpo)                 # w1 direct -> S2
            v.tensor_sub(S3_3, Rpo, S1_3)                 # r'-u -> S3
            sc.activation(S3[:], S3[:], AF.Ln)
            sc.activation(S3[:], S3[:], AF.Exp, scale=-1.0)   # 1/(r'-u)
            v.tensor_mul(S3_3, BQ3, S3_3)
            v.tensor_mul(S3_3, BQ3, S3_3)                 # alt = b^2/(r'-u) -> S3
            S1i = S1[:].bitcast(mybir.dt.int32)
            v.tensor_scalar(S1i, S1[:], 0.0, None, Alu.is_lt)    # mask u<0 -> S1
            v.copy_predicated(S2[:], S1i, S3[:])          # w1 -> S2
            v.tensor_mul(S1_3, Rpo, S2_3)                 # w2 = r'*w1 -> S1
            sc.activation(S3[:], S1[:], AF.Sqrt, scale=2.0)   # h = sqrt(2*w2) -> S3
            v.tensor_add(S1_3, S3_3, S2_3)                # den -> S1
            sc.activation(S3[:], S1[:], AF.Ln)
            sc.activation(S3[:], S3[:], AF.Exp, scale=-1.0)   # rden -> S3
            v.tensor_mul(S2_3, BQ3, S3_3)                 # q -> S2 (w1 dead)
            v.tensor_scalar(S2[:], S2[:], 1.02, -1.02, Alu.min, Alu.max)  # |q|<=1.02
            # theta0 chain (small, same ln/exp tables)
            sc.activation(GB, RMU0, AF.Ln)
            sc.activation(GB, GB, AF.Exp, scale=-1.0)     # 1/(r0-vx0)
            v.tensor_mul(GB, vy0, GB)
            v.tensor_mul(GB, vy0, GB)                     # alt0
            v.copy_predicated(W10, MSK0i, GB)             # w10
            v.tensor_mul(GB, r0, W10)
            sc.activation(GB, GB, AF.Ln, scale=2.0)
            sc.activation(GB, GB, AF.Exp, scale=0.5)      # h0
            v.tensor_add(GB, GB, W10)                     # den0
            sc.activation(GB, GB, AF.Ln)
            sc.activation(GB, GB, AF.Exp, scale=-1.0)
            v.tensor_mul(Q0, vy0, GB)                     # q0

            # trig table section (Arctan + Sin in trig_and_small).
            # Range reduction via magic-constant round-to-nearest:
            # f = y - round(y) in [-0.5, 0.5],  sin(2*pi*f) == sin(theta).
            sc.activation(A0, Q0, AF.Arctan)              # theta0/4
            sc.activation(S1_3, S2_3, AF.Arctan)          # A = delta/4 -> S1
            for b in range(NB):
                bs = slice(b * H, (b + 1) * H)
                v.tensor_tensor_scan(
                    S3[:, bs], S1[:, bs], S1[:, bs],
                    initial=A0[:, b:b + 1], op0=Alu.add, op1=Alu.bypass,
                )                                          # Theta -> S3
            MAGIC = float(1.5 * 2 ** 23)
            INV_HPI = float(2.0 / np.pi)                  # turns = Theta*4/(2*pi)
            v.tensor_scalar(S2[:], S3[:], INV_HPI, None, Alu.mult)       # yS
            v.tensor_scalar(S1[:], S2[:], MAGIC, -MAGIC, Alu.add, Alu.add)  # round
            v.tensor_sub(S2[:], S2[:], S1[:])             # fS in [-.5,.5]
            sc.activation(S2[:], S2[:], AF.Sin, scale=TWO_PI)   # sin -> S2
            v.tensor_scalar(S1[:], S3[:], INV_HPI, 0.25, Alu.mult, Alu.add)  # yC
            v.tensor_scalar(S3[:], S1[:], MAGIC, -MAGIC, Alu.add, Alu.add)
            v.tensor_sub(S1[:], S1[:], S3[:])             # fC
            sc.activation(S1[:], S1[:], AF.Sin, scale=TWO_PI)   # cos -> S1

            # C-section: velocities + positions, streamed out in 4 chunks of
            # 4 batch-columns through a double-buffered staging tile.
            # sin is in S2, cos in S1; S3 holds per-chunk vxs/vys (ping-pong).
            CB = 4                       # batch-columns per chunk
            CW = CB * H                  # 1024
            for ch in range(NB // CB):
                cols = slice(ch * CB, (ch + 1) * CB)
                OUTC = op.tile([P, CB * H * S], F32, tag="OUTC")
                OC4 = OUTC.rearrange("p (b t s) -> p b t s", b=CB, t=H)
                base = 2 * CW * (ch % 2)
                vxs = S3[:, base:base + CW]
                vys = S3[:, base + CW:base + 2 * CW]
                vxs3 = vxs.rearrange("p (b t) -> p b t", b=CB)
                vys3 = vys.rearrange("p (b t) -> p b t", b=CB)
                Rpo_c = RP3[:, cols, 1:HP]
                sin_c = S2_3[:, cols, :]
                cos_c = S1_3[:, cols, :]
                g.tensor_mul(OC4[:, :, :, 2], Rpo_c, cos_c)           # vx
                g.tensor_mul(OC4[:, :, :, 3], Rpo_c, sin_c)           # vy
                v.scalar_tensor_tensor(vxs3, cos_c, DT, Rpo_c, Alu.mult, Alu.mult)
                v.scalar_tensor_tensor(vys3, sin_c, DT, Rpo_c, Alu.mult, Alu.mult)
                for j in range(CB):
                    b = ch * CB + j
                    js = slice(j * H, (j + 1) * H)
                    v.tensor_tensor_scan(
                        OC4[:, j, :, 0], vxs[:, js], vxs[:, js],
                        initial=KX[:, b:b + 1], op0=Alu.add, op1=Alu.bypass,
                    )
                    v.tensor_tensor_scan(
                        OC4[:, j, :, 1], vys[:, js], vys[:, js],
                        initial=KY[:, b:b + 1], op0=Alu.add, op1=Alu.bypass,
                    )
                v.scalar_tensor_tensor(
                    OC4[:, :, :, 0], vxs3, -0.5, OC4[:, :, :, 0], Alu.mult, Alu.add
                )
                v.scalar_tensor_tensor(
                    OC4[:, :, :, 1], vys3, -0.5, OC4[:, :, :, 1], Alu.mult, Alu.add
                )
                # extra columns broadcast from initial_state (gpsimd)
                for k in range(4):
                    out_ap = bass.AP(
                        OUTC.tensor, 4 + k, [[CB * H * S, P], [H * S, CB], [S, H]]
                    )
                    in_ap = bass.AP(
                        IS.tensor, ch * CB * S + 4 + k,
                        [[NB * S, P], [S, CB], [0, H]],
                    )
                    g.tensor_copy(out_ap, in_ap)
                hw = CB * H * S // 2
                base_o = ch * CB * H * S
                sy.dma_start(
                    out=traj_r[:, base_o:base_o + hw], in_=OUTC[:, 0:hw]
                )
                sy.dma_start(
                    out=traj_r[:, base_o + hw:base_o + 2 * hw],
                    in_=OUTC[:, hw:2 * hw],
                )

    nc.finalize()
    if fixups:
        _split_multi_waits(nc)
    return nc


def _split_multi_waits(nc):
    """This toolchain's walrus embeds at most ONE sync-wait per instruction.
    Move all but the last wait of any multi-wait instruction onto NoOps
    inserted just before it (same engine, program order preserved).  Also
    drop the tail EVENT_SEMAPHORE_RANGE_CLEAR InstISA, whose raw encoding
    this walrus rejects ("ISA wrong length")."""
    n = 0
    for fn in nc.m.functions:
        for bb in fn.blocks:
            idx = 0
            while idx < len(bb.instructions):
                inst = bb.instructions[idx]
                if (
                    isinstance(inst, mybir.InstISA)
                    and getattr(inst, "op_name", "") == "EVENT_SEMAPHORE_RANGE_CLEAR"
                ):
                    del bb.instructions[idx]
                    continue
                si = getattr(inst, "sync_info", None)
                if si is not None and si.on_wait and len(si.on_wait) >= 2:
                    extra = list(si.on_wait[:-1])
                    keep = list(si.on_wait[-1:])
                    for w in extra:
                        nop = mybir.InstNoOp(
                            name=f"{inst.name}_wsplit{n}", ins=[], outs=[]
                        )
                        n += 1
                        nop.engine = inst.engine
                        nop.sync_info = mybir.SyncInfo(on_wait=[w], on_update=[])
                        bb.instructions.insert(idx, nop)
                        idx += 1
                    inst.sync_info = mybir.SyncInfo(
                        on_wait=keep, on_update=list(si.on_update)
                    )
                idx += 1
    return nc


def _get_built():
    global _BUILT
    if _BUILT is None:
        _BUILT = build_nc()
    return _BUILT


def kernel(initial_state: np.ndarray, actions: np.ndarray) -> np.ndarray:
    from concourse.bass_utils import run_bass_kernel_spmd

    nc = _get_built()
    in_maps = []
    for c in range(NCORES):
        sl = slice(c * BC, (c + 1) * BC)
        in_maps.append(
            {
                "initial_state": np.ascontiguousarray(initial_state[sl]),
                "actions": np.ascontiguousarray(actions[sl]),
            }
        )
    res = run_bass_kernel_spmd(nc, in_maps, core_ids=list(range(NCORES)))
    out = np.concatenate([r["traj"] for r in res.results], axis=0)
    return out


# revision 25
# speedup vs baseline: 10595.5472x; 10595.5472x over previous
"""Trainium2 Bass kernel for DifferentiablePointMassSimulator.

Math: the 2-D point-mass scan is reformulated in polar velocity coordinates.
With v = r*e^{i*theta}, a_t = DT*thrust, b_t = DT*torque:
    v' = e^{i*theta} * (r + a + i*b)
so the radius obeys a scalar recurrence independent of the angle:
    m_{t+1} = (m_t + (a^2+b^2)_t) + (2*a_t)*r_t,   r_t = sqrt(m_t)
and the angle increment delta_t = atan2(b_t, r_t + a_t) is computed post-hoc
from the radius sequence with the quarter-angle identity
    delta = 4*atan( b / (h + w1) ),  w1 = u + r',  u = r_t + a_t,  r' = r_{t+1}
    h = sqrt(2 * r' * w1)
whose atan argument always lies in [-1, 1] (ScalarE Arctan domain).
Near the delta ~ +-pi line (u < 0, |b| << |u|) the direct w1 = u + r' suffers
catastrophic cancellation; there we use the exact rationalization
    w1 = b^2 / (r' - u)        (since r'^2 - u^2 = b^2)
selected with copy_predicated on (u < 0).
theta_t = theta0 + cumsum(delta) via tensor_tensor_scan.  sin/cos via the
magic-constant round-to-nearest range reduction: with y = theta*2/pi (turns),
f = y - ((y + 1.5*2^23) - 1.5*2^23) lies in [-0.5, 0.5], and
sin(2*pi*f) = sin(theta) via the ScalarE Sin table (cos via y + 0.25).
Reciprocals are exp(-ln(x)) on ScalarE (custom DVE ops and the Reciprocal /
Rsqrt tables are unavailable in this toolchain).
Positions: pos_{t+1} = pos_t + DT*(v_t + v_{t+1})/2 exactly, so with
vxs_t = DT*vx_out[t]:
    px_out[t] = Cx_t - 0.5*vxs_t,  Cx = scan(+, vxs, init = px0 + DT*vx0/2).

Sharding: pure data parallel, batch 16384 -> 8 cores x 2048; on-core layout
batch = 128 partitions x 16 columns (b_local = p*16 + col).
"""

import sys

sys.path.insert(0, "/opt/trn_rl_repo")

import numpy as np

import concourse.bass as bass
import concourse.mybir as mybir
from concourse.tile import TileContext

DT = 1.0 / 30.0
P = 128          # partitions
NB = 16          # batch columns per partition
H = 256          # horizon
HP = H + 1
S = 8            # state dim
BC = P * NB      # batch per core (2048)
NCORES = 8
B = BC * NCORES

F32 = mybir.dt.float32
PI = float(np.pi)
TWO_PI = float(2.0 * np.pi)

_BUILT = None


def build_nc(fixups=True):
    Alu = mybir.AluOpType
    AF = mybir.ActivationFunctionType

    nc = bass.Bass()
    ist = nc.dram_tensor("initial_state", [BC, S], F32, kind="ExternalInput")
    act = nc.dram_tensor("actions", [BC, H, 2], F32, kind="ExternalInput")
    traj = nc.dram_tensor("traj", [BC, H, S], F32, kind="ExternalOutput")

    ist_r = ist.rearrange("(p q) s -> p (q s)", p=P)       # (128, 128)
    act_r = act.rearrange("(p q) h a -> p (q h a)", p=P)   # (128, 8192)
    traj_r = traj.rearrange("(p q) h s -> p (q h s)", p=P)  # (128, 32768)

    v = nc.vector
    g = nc.gpsimd
    sc = nc.scalar
    sy = nc.sync

    with TileContext(nc) as tc:
        with tc.tile_pool(name="pers", bufs=1) as pp, \
                tc.tile_pool(name="outc", bufs=2) as op:
            RP = pp.tile([P, NB * HP], F32, tag="RP")      # r_k at slot k
            A2 = pp.tile([P, NB * H], F32, tag="A2")       # 2*DT*thrust
            BQ = pp.tile([P, NB * H], F32, tag="BQ")       # DT*torque
            CARR = pp.tile([P, NB * H], F32, tag="CARR")   # a^2+b^2
            IS = pp.tile([P, NB * S], F32, tag="IS")
            # big tmps: 3 explicit rotating slots
            S1 = pp.tile([P, NB * H], F32, tag="S1")
            S2 = pp.tile([P, NB * H], F32, tag="S2")
            S3 = pp.tile([P, NB * H], F32, tag="S3")
            # small state tiles, packed into one allocation
            SMALL = pp.tile([P, NB * 12], F32, tag="SMALL")
            M = SMALL[:, 0 * NB:1 * NB]
            T1 = SMALL[:, 1 * NB:2 * NB]
            GA = SMALL[:, 2 * NB:3 * NB]   # scan scratch half 0
            GB = SMALL[:, 3 * NB:4 * NB]   # scan scratch half 1
            Q0 = SMALL[:, 4 * NB:5 * NB]
            A0 = SMALL[:, 5 * NB:6 * NB]
            KX = SMALL[:, 6 * NB:7 * NB]
            KY = SMALL[:, 7 * NB:8 * NB]
            W10 = SMALL[:, 8 * NB:9 * NB]
            RMU0 = SMALL[:, 9 * NB:10 * NB]
            MSK0 = SMALL[:, 10 * NB:11 * NB]

            # multi-dim views
            IS3 = IS.rearrange("p (b s) -> p b s", b=NB)
            RP3 = RP.rearrange("p (b k) -> p b k", b=NB)
            A23 = A2.rearrange("p (b t) -> p b t", b=NB)
            BQ3 = BQ.rearrange("p (b t) -> p b t", b=NB)
            C3 = CARR.rearrange("p (b t) -> p b t", b=NB)

            px0 = IS3[:, :, 0]
            py0 = IS3[:, :, 1]
            vx0 = IS3[:, :, 2]
            vy0 = IS3[:, :, 3]

            # ---------------- phase 0: loads + precompute ----------------
            sy.dma_start(out=IS[:], in_=ist_r[:])

            # actions -> A2, BQ, CARR (two 2MB chunks; squares on ScalarE)
            for hb in range(2):
                chunk = pp.tile([P, 8 * H * 2], F32, tag="S1" if hb == 0 else "S2")
                sy.dma_start(
                    out=chunk[:], in_=act_r[:, hb * 4096:(hb + 1) * 4096]
                )
                ch = chunk.rearrange("p (b t a) -> p b t a", b=8, t=H)
                thr = ch[:, :, :, 0]
                tor = ch[:, :, :, 1]
                bsl = slice(hb * 8, (hb + 1) * 8)
                v.tensor_scalar(A23[:, bsl, :], thr, 2.0 * DT, None, Alu.mult)
                v.tensor_scalar(BQ3[:, bsl, :], tor, DT, None, Alu.mult)
                sq = pp.tile([P, 8 * H], F32, tag="S3")
                sq3 = sq.rearrange("p (b t) -> p b t", b=8)
                sc.activation(sq3, thr, AF.Square, scale=DT)   # (DT*T)^2
                sq2 = pp.tile([P, 8 * H], F32, tag="S1" if hb == 1 else "S2")
                sq23 = sq2.rearrange("p (b t) -> p b t", b=8)
                sc.activation(sq23, tor, AF.Square, scale=DT)  # (DT*Q)^2
                v.tensor_add(C3[:, bsl, :], sq3, sq23)

            # r0, m0
            sc.activation(GA, vx0, AF.Square)
            sc.activation(GB, vy0, AF.Square)
            v.tensor_add(M, GA, GB)                      # m0 = r0^2
            sc.activation(RP3[:, :, 0], M, AF.Sqrt)      # r0
            r0 = RP3[:, :, 0]

            # theta0/4 prep: w10 = r0 + vx0, rationalized to vy0^2/(r0 - vx0)
            # when vx0 < 0.  All reciprocals are deferred to the ln/exp table
            # section after the scan (no custom DVE ops available).
            v.tensor_add(W10, r0, vx0)                   # w10 direct
            v.tensor_sub(RMU0, r0, vx0)                  # r0 - vx0
            MSK0i = MSK0.bitcast(mybir.dt.int32)
            v.tensor_scalar(MSK0i, vx0, 0.0, None, Alu.is_lt)  # mask vx0 < 0

            # pos cumsum seeds
            v.scalar_tensor_tensor(KX, vx0, DT / 2.0, px0, Alu.mult, Alu.add)
            v.scalar_tensor_tensor(KY, vy0, DT / 2.0, py0, Alu.mult, Alu.add)

            # ---------------- phase 1: radius scan ----------------
            # m' = (m + c_t) + (2 a_t) * r_t ; r_{t+1} = sqrt(m')
            # two staggered halves so ScalarE sqrt overlaps VectorE updates
            halves = [slice(0, 8), slice(8, 16)]
            Mh = [M[:, 0:8], M[:, 8:16]]
            T1h = [T1[:, 0:8], T1[:, 8:16]]
            Gh = [GA[:, 0:8], GB[:, 0:8]]
            for t in range(H):
                for hf in (0, 1):
                    v.tensor_add(T1h[hf], Mh[hf], C3[:, halves[hf], t])
                for hf in (0, 1):
                    v.tensor_mul(Gh[hf], A23[:, halves[hf], t], RP3[:, halves[hf], t])
                    v.tensor_add(Mh[hf], T1h[hf], Gh[hf])
                    sc.activation(RP3[:, halves[hf], t + 1], Mh[hf], AF.Sqrt)

            # ---------------- phase 2: angles, velocities, positions ------
            Rsh = RP3[:, :, 0:H]     # r_t
            Rpo = RP3[:, :, 1:HP]    # r_{t+1}
            S1_3 = S1.rearrange("p (b t) -> p b t", b=NB)
            S2_3 = S2.rearrange("p (b t) -> p b t", b=NB)
            S3_3 = S3.rearrange("p (b t) -> p b t", b=NB)

            # A-section: u, w1, w2, h, den, rden, q; w1 rationalized to
            # b^2/(r'-u) where u<0 (exact identity r'^2-u^2=b^2) to avoid
            # catastrophic cancellation near delta ~ +-pi.
            v.scalar_tensor_tensor(S1_3, A23, 0.5, Rsh, Alu.mult, Alu.add)   # u -> S1
            v.tensor_add(S2_3, S1_3, Rpo)                 # w1 direct -> S2
            v.tensor_sub(S3_3, Rpo, S1_3)                 # r'-u -> S3
            sc.activation(S3[:], S3[:], AF.Ln)
            sc.activation(S3[:], S3[:], AF.Exp, scale=-1.0)   # 1/(r'-u)
            v.tensor_mul(S3_3, BQ3, S3_3)
            v.tensor_mul(S3_3, BQ3, S3_3)                 # alt = b^2/(r'-u) -> S3
            S1i = S1[:].bitcast(mybir.dt.int32)
            v.tensor_scalar(S1i, S1[:], 0.0, None, Alu.is_lt)    # mask u<0 -> S1
            v.copy_predicated(S2[:], S1i, S3[:])          # w1 -> S2
            v.tensor_mul(S1_3, Rpo, S2_3)                 # w2 = r'*w1 -> S1
            sc.activation(S3[:], S1[:], AF.Sqrt, scale=2.0)   # h = sqrt(2*w2) -> S3
            v.tensor_add(S1_3, S3_3, S2_3)                # den -> S1
            sc.activation(S3[:], S1[:], AF.Ln)
            sc.activation(S3[:], S3[:], AF.Exp, scale=-1.0)   # rden -> S3
            v.tensor_mul(S2_3, BQ3, S3_3)                 # q -> S2 (w1 dead)
            v.tensor_scalar(S2[:], S2[:], 1.02, -1.02, Alu.min, Alu.max)  # |q|<=1.02
            # theta0 chain (small, same ln/exp tables)
            sc.activation(GB, RMU0, AF.Ln)
            sc.activation(GB, GB, AF.Exp, scale=-1.0)     # 1/(r0-vx0)
            v.tensor_mul(GB, vy0, GB)
            v.tensor_mul(GB, vy0, GB)                     # alt0
            v.copy_predicated(W10, MSK0i, GB)             # w10
            v.tensor_mul(GB, r0, W10)
            sc.activation(GB, GB, AF.Ln, scale=2.0)
            sc.activation(GB, GB, AF.Exp, scale=0.5)      # h0
            v.tensor_add(GB, GB, W10)                     # den0
            sc.activation(GB, GB, AF.Ln)
            sc.activation(GB, GB, AF.Exp, scale=-1.0)
            v.tensor_mul(Q0, vy0, GB)                     # q0

            # trig table section (Arctan + Sin in trig_and_small).
            # Range reduction via magic-constant round-to-nearest:
            # f = y - round(y) in [-0.5, 0.5],  sin(2*pi*f) == sin(theta).
            sc.activation(A0, Q0, AF.Arctan)              # theta0/4
            sc.activation(S1_3, S2_3, AF.Arctan)          # A = delta/4 -> S1
            for b in range(NB):
                bs = slice(b * H, (b + 1) * H)
                v.tensor_tensor_scan(
                    S3[:, bs], S1[:, bs], S1[:, bs],
                    initial=A0[:, b:b + 1], op0=Alu.add, op1=Alu.bypass,
                )                                          # Theta -> S3
            MAGIC = float(1.5 * 2 ** 23)
            INV_HPI = float(2.0 / np.pi)                  # turns = Theta*4/(2*pi)
            v.tensor_scalar(S2[:], S3[:], INV_HPI, None, Alu.mult)       # yS
            v.tensor_scalar(S1[:], S2[:], MAGIC, -MAGIC, Alu.add, Alu.add)  # round
            v.tensor_sub(S2[:], S2[:], S1[:])             # fS in [-.5,.5]
            sc.activation(S2[:], S2[:], AF.Sin, scale=TWO_PI)   # sin -> S2
            v.tensor_scalar(S1[:], S3[:], INV_HPI, 0.25, Alu.mult, Alu.add)  # yC
            v.tensor_scalar(S3[:], S1[:], MAGIC, -MAGIC, Alu.add, Alu.add)
            v.tensor_sub(S1[:], S1[:], S3[:])             # fC
            sc.activation(S1[:], S1[:], AF.Sin, scale=TWO_PI)   # cos -> S1

            # C-section: velocities + positions, streamed out in 4 chunks of
            # 4 batch-columns through a double-buffered staging tile.
            # sin is in S2, cos in S1; S3 holds per-chunk vxs/vys (ping-pong).
            CB = 4                       # batch-columns per chunk
            CW = CB * H                  # 1024
            for ch in range(NB // CB):
                cols = slice(ch * CB, (ch + 1) * CB)
                OUTC = op.tile([P, CB * H * S], F32, tag="OUTC")
                OC4 = OUTC.rearrange("p (b t s) -> p b t s", b=CB, t=H)
                base = 2 * CW * (ch % 2)
                vxs = S3[:, base:base + CW]
                vys = S3[:, base + CW:base + 2 * CW]
                vxs3 = vxs.rearrange("p (b t) -> p b t", b=CB)
                vys3 = vys.rearrange("p (b t) -> p b t", b=CB)
                Rpo_c = RP3[:, cols, 1:HP]
                sin_c = S2_3[:, cols, :]
                cos_c = S1_3[:, cols, :]
                g.tensor_mul(OC4[:, :, :, 2], Rpo_c, cos_c)           # vx
                g.tensor_mul(OC4[:, :, :, 3], Rpo_c, sin_c)           # vy
                v.scalar_tensor_tensor(vxs3, cos_c, DT, Rpo_c, Alu.mult, Alu.mult)
                v.scalar_tensor_tensor(vys3, sin_c, DT, Rpo_c, Alu.mult, Alu.mult)
                for j in range(CB):
                    b = ch * CB + j
                    js = slice(j * H, (j + 1) * H)
                    v.tensor_tensor_scan(
                        OC4[:, j, :, 0], vxs[:, js], vxs[:, js],
                        initial=KX[:, b:b + 1], op0=Alu.add, op1=Alu.bypass,
                    )
                    v.tensor_tensor_scan(
                        OC4[:, j, :, 1], vys[:, js], vys[:, js],
                        initial=KY[:, b:b + 1], op0=Alu.add, op1=Alu.bypass,
                    )
                v.scalar_tensor_tensor(
                    OC4[:, :, :, 0], vxs3, -0.5, OC4[:, :, :, 0], Alu.mult, Alu.add
                )
                v.scalar_tensor_tensor(
                    OC4[:, :, :, 1], vys3, -0.5, OC4[:, :, :, 1], Alu.mult, Alu.add
                )
                # extra columns broadcast from initial_state (gpsimd)
                for k in range(4):
                    out_ap = bass.AP(
                        OUTC.tensor, 4 + k, [[CB * H * S, P], [H * S, CB], [S, H]]
                    )
                    in_ap = bass.AP(
                        IS.tensor, ch * CB * S + 4 + k,
                        [[NB * S, P], [S, CB], [0, H]],
                    )
                    g.tensor_copy(out_ap, in_ap)
                hw = CB * H * S // 2
                base_o = ch * CB * H * S
                sy.dma_start(
                    out=traj_r[:, base_o:base_o + hw], in_=OUTC[:, 0:hw]
                )
                sy.dma_start(
                    out=traj_r[:, base_o + hw:base_o + 2 * hw],
                    in_=OUTC[:, hw:2 * hw],
                )

    nc.finalize()
    if fixups:
        _split_multi_waits(nc)
    return nc


def _split_multi_waits(nc):
    """This toolchain's walrus embeds at most ONE sync-wait per instruction.
    Move all but the last wait of any multi-wait instruction onto NoOps
    inserted just before it (same engine, program order preserved).  Also
    drop the tail EVENT_SEMAPHORE_RANGE_CLEAR InstISA, whose raw encoding
    this walrus rejects ("ISA wrong length")."""
    n = 0
    for fn in nc.m.functions:
        for bb in fn.blocks:
            idx = 0
            while idx < len(bb.instructions):
                inst = bb.instructions[idx]
                if (
                    isinstance(inst, mybir.InstISA)
                    and getattr(inst, "op_name", "") == "EVENT_SEMAPHORE_RANGE_CLEAR"
                ):
                    del bb.instructions[idx]
                    continue
                si = getattr(inst, "sync_info", None)
                if si is not None and si.on_wait and len(si.on_wait) >= 2:
                    extra = list(si.on_wait[:-1])
                    keep = list(si.on_wait[-1:])
                    for w in extra:
                        nop = mybir.InstNoOp(
                            name=f"{inst.name}_wsplit{n}", ins=[], outs=[]
                        )
                        n += 1
                        nop.engine = inst.engine
                        nop.sync_info = mybir.SyncInfo(on_wait=[w], on_update=[])
                        bb.instructions.insert(idx, nop)
                        idx += 1
                    inst.sync_info = mybir.SyncInfo(
                        on_wait=keep, on_update=list(si.on_update)
                    )
                idx += 1
    return nc


def _get_built():
    global _BUILT
    if _BUILT is None:
        _BUILT = build_nc()
    return _BUILT


def kernel(initial_state: np.ndarray, actions: np.ndarray) -> np.ndarray:
    from concourse.bass_utils import run_bass_kernel_spmd

    nc = _get_built()
    in_maps = []
    for c in range(NCORES):
        sl = slice(c * BC, (c + 1) * BC)
        in_maps.append(
            {
                "initial_state": np.ascontiguousarray(initial_state[sl]),
                "actions": np.ascontiguousarray(actions[sl]),
            }
        )
    res = run_bass_kernel_spmd(nc, in_maps, core_ids=list(range(NCORES)))
    out = np.concatenate([r["traj"] for r in res.results], axis=0)
    return out
